# revision 7
# baseline (speedup 1.0000x reference)
"""BERT+CRF loss (torchcrf-style, reduction=sum) on 8 Trainium2 NeuronCores.

Strategy (pure data parallel, batch sharded 8 ways, 8 sequences per core):
  X is quantized to fp8-e4m3 on the host (4x less HBM traffic than f32) and
  streamed once through TensorE with DoubleRow fp8 matmuls (256-deep k-tiles)
  to produce emissions^T [9, 512] per sequence.  The CRF forward recurrence
  is reformulated in exp space: step matrix M_t[i,j] = expT[i,j] * E_t[j]
  with E_t = exp(em_t + b).  Triples of steps (t = 3q+2, 3q+3, 3q+4) collapse
  into one 9x9 transfer matrix via a bilinear host constant G6 [81, 81]:
      T_q[i,j] = sum_{k,l} expT[i,k] Ea[k] expT[k,l] Eb[l] expT[l,j]
  so  M_{3q+2} M_{3q+3} M_{3q+4} = T_q * diag(E_{3q+4}).
  The replicated log-space outer sum (ema[k]+emb[l]) is built with two
  accumulating indicator matmuls and exponentiated in one activation; a
  second matmul against G6 yields T^T [81, 170] per sequence.  The device
  ships the 170 T_q matrices (bf16) plus strided exp(em) columns; the host
  multiplies the 9x9 chain in f64 with an order-preserving normalized tree
  reduce (O(B*170*81) work) and adds the label-indexed numerator terms
  (accumulated on-device via a masked-sum op on the GpSimd engine).
  The per-sequence stages are software-pipelined with a skew of 2 so
  TensorE always has independent DoubleRow work queued.
"""

import os
import sys

if "/opt/trn_rl_repo" not in sys.path:
    sys.path.insert(0, "/opt/trn_rl_repo")

import ml_dtypes
import numpy as np

B, S, H, L = 64, 512, 768, 9
NCORES = 8
BPC = B // NCORES          # sequences per core
LL = L * L                 # 81
NT = 170                   # triples per sequence: steps t=2..511; t=1 on host
NE = 171                   # exp(em) column pairs shipped: cols 3q'+{0,1}
HC = H // 128              # 6 h-chunks of 128
NKT = 3                    # DoubleRow k-tiles (256-deep each)
MP = 16                    # DoubleRow needs >=16 weight cols per plane; 9 padded
SCALE_W = 64.0             # W is scaled into fp8 range; exp() unscales

_CACHE = {}


def _build_bass():
    import concourse.bass as bass
    import concourse.bacc as bacc
    import concourse.mybir as mybir
    import concourse.tile as tile
    from contextlib import ExitStack

    f32 = mybir.dt.float32
    bf16 = mybir.dt.bfloat16
    f8 = mybir.dt.float8e4
    Alu = mybir.AluOpType
    Act = mybir.ActivationFunctionType
    DR = mybir.MatmulPerfMode.DoubleRow
    NP = BPC // 2              # sequence pairs per core

    nc = bacc.Bacc()

    # ---- I/O ----
    x8_d = nc.dram_tensor("x8", [BPC, 128, HC, S], f8, kind="ExternalInput")
    w8_d = nc.dram_tensor("w8", [128, HC, MP], f8, kind="ExternalInput")
    lab_d = nc.dram_tensor("lab9", [L, BPC, S], bf16, kind="ExternalInput")
    gp_d = nc.dram_tensor("Gpack", [LL, 3 * LL], bf16, kind="ExternalInput")
    cf_d = nc.dram_tensor("Cf32", [LL, 3], f32, kind="ExternalInput")

    t_out = nc.dram_tensor("t_out", [LL, BPC, NT], bf16, kind="ExternalOutput")
    eme_out = nc.dram_tensor("eme_out", [L, BPC, NE, 2], bf16, kind="ExternalOutput")
    nt_out = nc.dram_tensor("nt_out", [L, NP], f32, kind="ExternalOutput")

    with ExitStack() as ctx:
        tc = ctx.enter_context(tile.TileContext(nc))
        const = ctx.enter_context(tc.tile_pool(name="const", bufs=1))
        xpool = ctx.enter_context(tc.tile_pool(name="x", bufs=2))
        epool = ctx.enter_context(tc.tile_pool(name="e", bufs=2))
        spool = ctx.enter_context(tc.tile_pool(name="sm", bufs=2))
        upool = ctx.enter_context(tc.tile_pool(name="u", bufs=2))
        rpool = ctx.enter_context(tc.tile_pool(name="res", bufs=1))
        ps_em = ctx.enter_context(tc.tile_pool(name="psem", bufs=2, space="PSUM"))
        ps_rep = ctx.enter_context(tc.tile_pool(name="psrep", bufs=2, space="PSUM"))
        ps_g6 = ctx.enter_context(tc.tile_pool(name="psg6", bufs=2, space="PSUM"))

        # ---- constants into SBUF (two packed DMAs + weights + labels) ----
        w8_sb = const.tile([128, HC, MP], f8)
        nc.sync.dma_start(w8_sb[:], w8_d[:])
        gp_sb = const.tile([LL, 3 * LL], bf16)
        nc.sync.dma_start(gp_sb[:], gp_d[:])
        cf_sb = const.tile([LL, 3], f32)
        nc.scalar.dma_start(cf_sb[:], cf_d[:])
        lab_sb = const.tile([L, BPC, S], bf16)
        nc.scalar.dma_start(lab_sb[:], lab_d[:])
        g6_ap = gp_sb[:, 0:LL]
        ra_ap = gp_sb[0:L, LL : 2 * LL]
        rb_ap = gp_sb[0:L, 2 * LL : 3 * LL]
        bias81 = cf_sb[:, 0:1]
        iota_ap = cf_sb[0:L, 1:2]
        bias9 = cf_sb[0:L, 2:3]

        # ---- persistent result collect tiles ----
        coll_sb = rpool.tile([LL, BPC, NT], bf16)
        eme_sb = rpool.tile([L, BPC, NE, 2], bf16)
        ntag_sb = rpool.tile([L, NP], f32)

        xts = [None] * NP
        emps = [None] * NP
        emsb = [None] * NP
        usbs = [None] * NP
        g6ps = [None] * NP

        def stage_a(p):
            # stream 2 sequences' X^T (fp8) in one DMA
            xt = xpool.tile([128, 2, HC, S], f8)
            nc.sync.dma_start(xt[:], x8_d[2 * p : 2 * p + 2].rearrange("b p c s -> p b c s"))
            xts[p] = xt
            # emissions^T [16, 2, S]: 3 DoubleRow k-tiles per sequence
            em_ps = ps_em.tile([MP, 2, S], f32)
            for i in range(2):
                for t in range(NKT):
                    nc.tensor.matmul(
                        em_ps[:, i],
                        w8_sb[:, 2 * t : 2 * t + 2, :],
                        xt[:, i, 2 * t : 2 * t + 2, :],
                        start=(t == 0), stop=(t == NKT - 1),
                        perf_mode=DR,
                    )
            emps[p] = em_ps

        def stage_b(p):
            em_ps = emps[p]
            # scaled emissions to SBUF (bf16) for the replicate matmuls
            em_sb = epool.tile([L, 2, S], bf16)
            nc.vector.tensor_copy(em_sb[:], em_ps[0:L])
            emsb[p] = em_sb
            # exp(em) at columns 3q'+{0,1}: v0 (col 0), host M_1 step (col 1),
            # triple diag factors (cols 4,7,...,511); one ACT per sequence
            for i in range(2):
                eme_ap = bass.AP(
                    em_ps.tensor, em_ps[:, i].offset,
                    [[em_ps[:].ap[0][0], L], [3, NE], [1, 2]],
                )
                nc.scalar.activation(
                    eme_sb[:, 2 * p + i], eme_ap, Act.Exp,
                    bias=bias9, scale=1.0 / SCALE_W,
                )
            # numerator: sum_t em_scaled[label_t, t], pair-accumulated
            msk = spool.tile([L, 2, S], bf16)
            nc.vector.scalar_tensor_tensor(
                out=msk[:], in0=lab_sb[:, 2 * p : 2 * p + 2], scalar=iota_ap,
                in1=em_sb[:], op0=Alu.is_equal, op1=Alu.mult,
                accum_out=ntag_sb[:, p : p + 1],
            )

        def stage_c1(p):
            em_sb = emsb[p]
            # rep[(k,l), (i,q)] = ema[k, i, 3q+2] + emb[l, i, 3q+3] via two
            # accumulating indicator matmuls over the 340-wide pair view
            ap0 = em_sb[:].ap[0]
            sv = S * 1  # bank stride in elements within em_sb free dims
            ea_ap = bass.AP(
                em_sb.tensor, em_sb[:].offset + 2, [[ap0[0], L], [sv, 2], [3, NT]]
            )
            eb_ap = bass.AP(
                em_sb.tensor, em_sb[:].offset + 3, [[ap0[0], L], [sv, 2], [3, NT]]
            )
            rep_ps = ps_rep.tile([LL, 2, NT], f32)
            nc.tensor.matmul(rep_ps[:], ra_ap, ea_ap, start=True, stop=False)
            nc.tensor.matmul(rep_ps[:], rb_ap, eb_ap, start=False, stop=True)
            return rep_ps

        def stage_c2(p, rep_ps):
            u_sb = upool.tile([LL, 2, NT], bf16)
            nc.scalar.activation(
                u_sb[:], rep_ps[:], Act.Exp, bias=bias81, scale=1.0 / SCALE_W
            )
            usbs[p] = u_sb

        def stage_c3(p):
            # T^T [(i,j), (i,q)] = G6^T @ u  (pair of triple-transfer matrices)
            t_ps = ps_g6.tile([LL, 2, NT], f32)
            nc.tensor.matmul(t_ps[:], g6_ap, usbs[p][:], start=True, stop=True)
            g6ps[p] = t_ps

        def stage_c4(p):
            nc.vector.tensor_copy(coll_sb[:, 2 * p : 2 * p + 2], g6ps[p][:])
            xts[p] = emps[p] = emsb[p] = usbs[p] = g6ps[p] = None

        stage_a(0)
        stage_a(1)
        stage_b(0)
        stage_c2(0, stage_c1(0))
        for p in range(1, NP):
            if p + 1 < NP:
                stage_a(p + 1)
            stage_b(p)
            stage_c3(p - 1)
            stage_c4(p - 1)
            stage_c2(p, stage_c1(p))
            if p == 2:
                # first-half outputs overlap with second-half compute
                nc.sync.dma_start(t_out[:, 0:4], coll_sb[:, 0:4])
                nc.scalar.dma_start(eme_out[:, 0:4], eme_sb[:, 0:4])
        stage_c3(NP - 1)
        stage_c4(NP - 1)

        nc.sync.dma_start(t_out[:, 4:BPC], coll_sb[:, 4:BPC])
        nc.scalar.dma_start(eme_out[:, 4:BPC], eme_sb[:, 4:BPC])
        nc.scalar.dma_start(nt_out[:], ntag_sb[:])

    if not nc.is_finalized():
        nc.finalize()
    return nc


def _get_nc():
    if "nc" not in _CACHE:
        _CACHE["nc"] = _build_bass()
    return _CACHE["nc"]


def _host_consts(trans, bb):
    expT = np.exp(trans.astype(np.float64))                      # [9,9] f64
    r = np.arange(LL)
    c = np.arange(LL)
    k = r // L
    l = r % L
    i = c // L
    j = c % L
    gpack = np.zeros((LL, 3 * LL), dtype=ml_dtypes.bfloat16)
    # G6[(k,l), (i,j)] = expT[i,k] * expT[k,l] * expT[l,j]
    gpack[:, 0:LL] = (
        expT[i[None, :], k[:, None]]
        * expT[k[:, None], l[:, None]]
        * expT[l[:, None], j[None, :]]
    ).astype(ml_dtypes.bfloat16)
    gpack[0:L, LL : 2 * LL] = k[None, :] == np.arange(L)[:, None]
    gpack[0:L, 2 * LL : 3 * LL] = l[None, :] == np.arange(L)[:, None]
    b64 = bb.astype(np.float64)
    cf = np.zeros((LL, 3), dtype=np.float32)
    cf[:, 0] = b64[k] + b64[l]
    cf[0:L, 1] = np.arange(L)
    cf[0:L, 2] = bb
    return expT, gpack, cf


def _numpy_reference(hs, mask, labels, W, bb, st, en, tr):
    # general fallback (only used when attention_mask is not all ones)
    em = hs.astype(np.float64) @ W.astype(np.float64) + bb.astype(np.float64)
    maskb = mask.astype(bool)
    maskf = mask.astype(np.float64)
    em_tag = np.take_along_axis(em, labels[..., None], axis=-1)[..., 0]
    num = st.astype(np.float64)[labels[:, 0]] + em_tag[:, 0]
    trs = tr.astype(np.float64)[labels[:, :-1], labels[:, 1:]]
    num = num + np.sum((trs + em_tag[:, 1:]) * maskf[:, 1:], axis=1)
    last = mask.sum(axis=1).astype(np.int64) - 1
    num = num + en.astype(np.float64)[labels[np.arange(len(labels)), last]]
    alpha = st.astype(np.float64)[None, :] + em[:, 0]
    for t in range(1, em.shape[1]):
        x = alpha[:, :, None] + tr.astype(np.float64)[None, :, :] + em[:, t][:, None, :]
        m = x.max(axis=1, keepdims=True)
        nxt = np.log(np.exp(x - m).sum(axis=1)) + m[:, 0, :]
        alpha = np.where(maskb[:, t][:, None], nxt, alpha)
    x = alpha + en.astype(np.float64)[None, :]
    m = x.max(axis=1, keepdims=True)
    denom = np.log(np.exp(x - m).sum(axis=1)) + m[:, 0]
    return np.asarray((denom - num).sum(), dtype=np.float32)


def _run_device(nc, in_maps):
    if os.environ.get("KERNEL_SIM"):
        from concourse.bass_interp import MultiCoreSim

        sim = MultiCoreSim(nc, len(in_maps))
        for t, m in enumerate(in_maps):
            for k2, v in m.items():
                sim.cores[t].tensor(k2)[:] = v
        sim.simulate()
        outs = []
        for t in range(len(in_maps)):
            outs.append(
                {
                    name: np.array(sim.cores[t].tensor(name))
                    for name in ("t_out", "eme_out", "nt_out")
                }
            )

        class _R:
            results = outs
            exec_time_ns = None

        return _R()
    from concourse import bass_utils

    return bass_utils.run_bass_kernel_spmd(nc, in_maps, list(range(len(in_maps))))


def kernel(**inputs):
    hs = np.asarray(inputs["hidden_states"], dtype=np.float32)
    mask = np.asarray(inputs["attention_mask"])
    labels = np.asarray(inputs["labels"]).astype(np.int64)
    W = np.asarray(inputs["W"], dtype=np.float32)
    bb = np.asarray(inputs["b"], dtype=np.float32)
    st = np.asarray(inputs["start_trans"], dtype=np.float32)
    en = np.asarray(inputs["end_trans"], dtype=np.float32)
    tr = np.asarray(inputs["trans"], dtype=np.float32)

    if not np.all(mask == 1):
        return _numpy_reference(hs, mask, labels, W, bb, st, en, tr)

    expT64, gpack, cf = _host_consts(tr, bb)

    # X -> fp8 e4m3 in [B, 128, HC, S] layout (h = 128*c + p)
    xq = np.clip(hs, -224.0, 224.0).astype(ml_dtypes.float8_e4m3)   # [B, S, H]
    x8 = np.ascontiguousarray(
        xq.transpose(0, 2, 1).reshape(B, HC, 128, S).transpose(0, 2, 1, 3)
    )                                                               # [B, 128, HC, S]
    wpad = np.zeros((H, MP), dtype=np.float32)
    wpad[:, :L] = W * SCALE_W
    w8 = np.ascontiguousarray(
        np.clip(wpad, -224.0, 224.0)
        .astype(ml_dtypes.float8_e4m3)
        .reshape(HC, 128, MP)
        .transpose(1, 0, 2)
    )                                                               # [128, HC, MP]
    lab9 = np.ascontiguousarray(
        np.broadcast_to(
            labels.astype(ml_dtypes.bfloat16).reshape(NCORES, 1, BPC, S),
            (NCORES, L, BPC, S),
        )
    )                                                               # [NC, 9, BPC, S]
    nc = _get_nc()
    in_maps = []
    for k in range(NCORES):
        sl = slice(k * BPC, (k + 1) * BPC)
        in_maps.append(
            {
                "x8": x8[sl],
                "w8": w8,
                "lab9": lab9[k],
                "Gpack": gpack,
                "Cf32": cf,
            }
        )
    res = _run_device(nc, in_maps)
    _CACHE["last_results"] = res

    # ---- host combine (f64, O(B * NT * 81)) ----
    st64 = st.astype(np.float64)
    en64 = en.astype(np.float64)
    e_en = np.exp(en64)
    e_st = np.exp(st64)
    total = 0.0
    for k in range(NCORES):
        r = res.results[k]
        Tm = (
            r["t_out"].astype(np.float64)
            .reshape(L, L, BPC, NT)
            .transpose(2, 3, 0, 1)
        )                                                   # [b, q, i, j]
        emE = r["eme_out"].astype(np.float64)               # [9, b, NE, 2]
        D = emE[:, :, 1:, 1].transpose(1, 2, 0)             # [b, q, j] diag factors
        M = Tm * D[:, :, None, :]
        logacc = np.zeros(BPC)
        while M.shape[1] > 1:
            n = M.shape[1]
            half = n // 2
            P = M[:, 0 : 2 * half : 2] @ M[:, 1 : 2 * half : 2]
            if n % 2:
                P = np.concatenate([P, M[:, 2 * half :]], axis=1)
            m = P.max(axis=(2, 3), keepdims=True)
            P /= m
            logacc += np.log(m[:, :, 0, 0]).sum(axis=1)
            M = P
        v0 = e_st[:, None] * emE[:, :, 0, 0]                # [9, b]
        v1 = (v0.T @ expT64) * emE[:, :, 0, 1].T            # host M_1 step [b, 9]
        v = np.einsum("bj,bjk->bk", v1, M[:, 0])
        denom = np.log(v @ e_en) + logacc
        total += float(denom.sum())
        total -= float(r["nt_out"].astype(np.float64).sum()) / SCALE_W
        lb = labels[k * BPC : (k + 1) * BPC]
        total -= float(
            st64[lb[:, 0]].sum()
            + en64[lb[:, -1]].sum()
            + tr.astype(np.float64)[lb[:, :-1], lb[:, 1:]].sum()
            + bb.astype(np.float64)[lb].sum()
        )
    return np.asarray(total, dtype=np.float32)
